# revision 23
# baseline (speedup 1.0000x reference)
"""Causal self-attention (GPT-2 style) on 8 Trainium2 NeuronCores.

Sharding: tensor-parallel over heads. Each of the 8 cores owns 2 of the 16
heads: it computes the q/k/v projections for its heads (column-sharded
w_attn), runs causal attention for them, and multiplies by its row-slice of
w_proj, producing a partial (B*T, E) output. The host sums the 8 partials.

Layout trick: the host feeds X pre-transposed (xT = X.T, [E, B*T]) so every
matmul on-device has its contraction dim on partitions with no on-device
transposes of X. Attention is computed in the S^T = K @ Q^T layout
([s, ti] tiles): softmax denominators come for free from a ones-column
appended to V (row 64 of the AV accumulator), and normalization is applied
to the 64-row attn-out^T slabs. exp() is applied without max-subtraction:
scores for this distribution are O(1) (softmax is shift-invariant; the
reference's masked lanes underflow to exactly 0 the same way). Causal
masking adds -1e9 (underflowing exp to +0) via a wide sliding-window mask
so each E tile has a single producer engine.

Matmuls use float32r operands (full-rate on the PE at N>=256 vs 4x slower
plain fp32); operand tensors are declared float32r end-to-end so DMAs are
passthrough and compute producers round on write. Built on bacc.Bacc +
compile() so multi-wait instructions get legalized (generate_event_semaphores
splits them; raw Bass hits walrus "Too many sync wait commands").
"""

import numpy as np
from contextlib import ExitStack

import concourse.bass as bass
import concourse.bacc as bacc
import concourse.mybir as mybir
import concourse.tile as tile
from concourse import bass_utils

F32 = mybir.dt.float32
F32R = mybir.dt.float32r
AF = mybir.ActivationFunctionType

B, T, E = 2, 2048, 1024
NH, DH = 16, 64
NCORES = 8
HPC = NH // NCORES          # heads per core = 2
BT = B * T                  # 4096 tokens total
TCH = 512                   # token chunk (matmul moving size / PSUM bank)
NTC = BT // TCH             # 8 token chunks
NE = E // 128               # 8 contraction tiles over E
ST = T // 128               # 16 key tiles per batch
CPB = T // TCH              # 4 query chunks per batch
SCALE = 1.0 / 8.0           # 1/sqrt(DH)


def _kernel_body(ctx: ExitStack, tc: tile.TileContext, yT, xT, wqkv, bqkv,
                 wproj, bproj, maskd, identd, onesd):
    nc = tc.nc

    singles = ctx.enter_context(tc.tile_pool(name="singles", bufs=1))
    xpool = ctx.enter_context(tc.tile_pool(name="xpool", bufs=2))
    vtp = ctx.enter_context(tc.tile_pool(name="vtp", bufs=2))
    epool = ctx.enter_context(tc.tile_pool(name="epool", bufs=3))
    rpool = ctx.enter_context(tc.tile_pool(name="rpool", bufs=2))
    ypool = ctx.enter_context(tc.tile_pool(name="ypool", bufs=3))
    psum = ctx.enter_context(tc.tile_pool(name="psum", space="PSUM", bufs=2))

    # --- constants / persistent buffers ---
    wsb = singles.tile([128, NE, 3 * 128], F32R)
    nc.sync.dma_start(out=wsb, in_=wqkv.rearrange("(e p) m -> p e m", p=128))
    bq_sb = singles.tile([128, 3], F32)
    nc.sync.dma_start(out=bq_sb, in_=bqkv.rearrange("(c p) -> p c", p=128))
    wp_sb = singles.tile([128, E], F32R)
    nc.sync.dma_start(out=wp_sb, in_=wproj)
    bp_sb = singles.tile([128, NE], F32)
    nc.sync.dma_start(out=bp_sb, in_=bproj.rearrange("(c p) -> p c", p=128))
    mask_sb = singles.tile([128, 2 * TCH], F32)
    nc.sync.dma_start(out=mask_sb, in_=maskd)
    # stacked identity: rows 0-63 = I64, rows 64-127 = I64, so a slice with
    # any 64-aligned base partition is available for PE transposes
    id_sb = singles.tile([128, 64], F32)
    nc.sync.dma_start(out=id_sb, in_=identd)

    ones_sb = singles.tile([1, 64], F32R)
    nc.sync.dma_start(out=ones_sb, in_=onesd[0:1, :])

    qT = singles.tile([128, BT], F32R)   # rows: 2 heads x 64 dh
    kT = singles.tile([128, BT], F32R)
    aoT = singles.tile([128, BT], F32R)  # normalized attn-out^T
    # V in natural [s, dh] layout per (batch, head, s-tile), with a ones
    # column at index 64 (computes the softmax denominator inside AV).
    v1 = singles.tile([128, B, HPC, ST, 65], F32R)
    nc.sync.dma_start(
        out=v1[:, :, :, :, 64:65],
        in_=onesd.rearrange("p (b h s) -> p b h s", b=B, h=HPC)[:, :, :, :, None])

    # --- phase A: qkv^T = wqkv^T @ x^T, plus V tile transposes ---
    for tcx in range(NTC):
        bidx = tcx // CPB
        xch = xpool.tile([128, NE, TCH], F32R)
        nc.sync.dma_start(
            out=xch,
            in_=xT.rearrange("(e p) t -> p e t", p=128)[
                :, :, tcx * TCH:(tcx + 1) * TCH],
        )
        for m in range(3):
            ps = psum.tile([128, TCH], F32, tag="mm512", bufs=3, name=f"psA{tcx}_{m}")
            for e in range(NE):
                nc.tensor.matmul(
                    ps,
                    lhsT=wsb[:, e, m * 128:(m + 1) * 128],
                    rhs=xch[:, e, :],
                    start=(e == 0),
                    stop=(e == NE - 1),
                )
            if m == 0:
                nc.vector.tensor_scalar_add(
                    qT[:, tcx * TCH:(tcx + 1) * TCH], ps, bq_sb[:, 0:1])
            elif m == 1:
                nc.vector.tensor_scalar_add(
                    kT[:, tcx * TCH:(tcx + 1) * TCH], ps, bq_sb[:, 1:2])
            else:
                vtile = vtp.tile([128, TCH], F32)
                nc.scalar.activation(vtile, ps, AF.Identity, bias=bq_sb[:, 2:3])
                for hh in range(HPC):
                    for ss in range(TCH // 128):
                        s_idx = (tcx % CPB) * (TCH // 128) + ss
                        ps_t = psum.tile([128, 64], F32, tag="aux", bufs=2,
                                         name=f"pst{tcx}_{hh}_{ss}")
                        nc.tensor.transpose(
                            ps_t,
                            vtile[hh * 64:(hh + 1) * 64, ss * 128:(ss + 1) * 128],
                            id_sb[hh * 64:(hh + 1) * 64, :],
                        )
                        nc.scalar.copy(
                            v1[:, bidx, hh, s_idx, 0:64], ps_t)

    # --- phase B: causal attention per (batch, head) in S^T layout ---
    for bidx in range(B):
        for hh in range(HPC):
            hs = slice(hh * 64, (hh + 1) * 64)
            for c in range(CPB):
                tis = slice(bidx * T + c * TCH, bidx * T + (c + 1) * TCH)
                out_ps = psum.tile([65, TCH], F32, tag="out65", bufs=2,
                                   name=f"outp{bidx}_{hh}_{c}")
                smax = 4 * c + 3
                for s in range(smax + 1):
                    s_ps = psum.tile([128, TCH], F32, tag="mm512", bufs=3,
                                     name=f"psS{bidx}_{hh}_{c}_{s}")
                    nc.tensor.matmul(
                        s_ps,
                        lhsT=kT[hs, bidx * T + s * 128:bidx * T + (s + 1) * 128],
                        rhs=qT[hs, tis],
                        start=True, stop=True,
                    )
                    et = epool.tile([128, TCH], F32R)
                    if s >= 4 * c:
                        # additive causal mask (-1e9 where ti < s) in PSUM,
                        # then one exp: E keeps a single producer engine
                        off = s * 128 - c * TCH
                        nc.vector.tensor_add(
                            s_ps, s_ps, mask_sb[:, TCH - off:2 * TCH - off])
                    nc.scalar.activation(et, s_ps, AF.Exp, scale=SCALE)
                    nc.tensor.matmul(
                        out_ps,
                        lhsT=v1[:, bidx, hh, s, :],
                        rhs=et,
                        start=(s == 0), stop=(s == smax),
                    )
                # normalize: rows 0..63 are unnormalized out^T, row 64 = denom
                r = rpool.tile([1, TCH], F32R)
                nc.vector.reciprocal(r, out_ps[64:65, :])
                rb_ps = psum.tile([64, TCH], F32, tag="aux", bufs=2,
                                  name=f"rbp{bidx}_{hh}_{c}")
                nc.tensor.matmul(rb_ps, lhsT=ones_sb,
                                 rhs=r, start=True, stop=True)
                rbs = rpool.tile([64, TCH], F32)
                nc.scalar.copy(rbs, rb_ps)
                nc.vector.tensor_mul(aoT[hs, tis], out_ps[0:64, :], rbs)

    # --- phase C: partial y^T = wproj^T @ attn_out^T (+ bias on core 0) ---
    for oc in range(NE):
        for tc2 in range(NTC):
            ps = psum.tile([128, TCH], F32, tag="mm512", bufs=3,
                           name=f"psC{oc}_{tc2}")
            nc.tensor.matmul(
                ps,
                lhsT=wp_sb[:, oc * 128:(oc + 1) * 128],
                rhs=aoT[:, tc2 * TCH:(tc2 + 1) * TCH],
                start=True, stop=True,
            )
            ysb = ypool.tile([128, TCH], F32)
            nc.scalar.activation(ysb, ps, AF.Identity, bias=bp_sb[:, oc:oc + 1])
            nc.sync.dma_start(
                out=yT[oc * 128:(oc + 1) * 128, tc2 * TCH:(tc2 + 1) * TCH],
                in_=ysb)


def build_bass():
    nc = bacc.Bacc("TRN2", target_bir_lowering=False, debug=False,
                   enable_asserts=False, num_devices=NCORES)
    xT = nc.dram_tensor("xT", [E, BT], F32R, kind="ExternalInput").ap()
    wqkv = nc.dram_tensor("wqkv", [E, 3 * 128], F32R, kind="ExternalInput").ap()
    bqkv = nc.dram_tensor("bqkv", [3 * 128], F32, kind="ExternalInput").ap()
    wproj = nc.dram_tensor("wproj", [128, E], F32R, kind="ExternalInput").ap()
    bproj = nc.dram_tensor("bproj", [E], F32, kind="ExternalInput").ap()
    maskd = nc.dram_tensor("maskd", [128, 2 * TCH], F32, kind="ExternalInput").ap()
    identd = nc.dram_tensor("identd", [128, 64], F32, kind="ExternalInput").ap()
    onesd = nc.dram_tensor("onesd", [128, 64], F32R, kind="ExternalInput").ap()
    yT = nc.dram_tensor("yT", [E, BT], F32, kind="ExternalOutput").ap()
    with tile.TileContext(nc) as tc:
        with nc.allow_low_precision(reason="fp32r matmul operand production"):
            with ExitStack() as ctx:
                _kernel_body(ctx, tc, yT, xT, wqkv, bqkv, wproj, bproj, maskd,
                             identd, onesd)
    nc.compile()
    return nc


def make_in_maps(inputs):
    stacked = np.asarray(inputs["stacked"], dtype=np.float32)
    w_attn = np.asarray(inputs["w_attn"], dtype=np.float32)
    b_attn = np.asarray(inputs["b_attn"], dtype=np.float32)
    w_proj = np.asarray(inputs["w_proj"], dtype=np.float32)
    b_proj = np.asarray(inputs["b_proj"], dtype=np.float32)

    xT = np.ascontiguousarray(stacked.reshape(BT, E).T)
    # W[r, w] = 0 where (w - TCH) >= r else -1e9; sliced per diagonal offset
    ww = np.arange(2 * TCH)[None, :] - TCH
    rr = np.arange(128)[:, None]
    mask = np.where(ww >= rr, 0.0, -1e9).astype(np.float32)
    ident = np.concatenate(
        [np.eye(64, dtype=np.float32), np.eye(64, dtype=np.float32)], axis=0)

    in_maps = []
    for c in range(NCORES):
        lo = c * HPC * DH
        hi = lo + HPC * DH
        wq = np.concatenate(
            [w_attn[:, lo:hi], w_attn[:, E + lo:E + hi],
             w_attn[:, 2 * E + lo:2 * E + hi]], axis=1)
        bq = np.concatenate(
            [b_attn[lo:hi], b_attn[E + lo:E + hi], b_attn[2 * E + lo:2 * E + hi]])
        in_maps.append({
            "xT": xT,
            "wqkv": np.ascontiguousarray(wq),
            "bqkv": np.ascontiguousarray(bq),
            "wproj": np.ascontiguousarray(w_proj[lo:hi, :]),
            "bproj": b_proj if c == 0 else np.zeros_like(b_proj),
            "maskd": mask,
            "identd": ident,
            "onesd": np.ones((128, 64), dtype=np.float32),
        })
    return in_maps


_NC = None


def _get_nc():
    global _NC
    if _NC is None:
        _NC = build_bass()
    return _NC


def run(inputs, trace=False):
    nc = _get_nc()
    in_maps = make_in_maps(inputs)
    res = bass_utils.run_bass_kernel_spmd(
        nc, in_maps, core_ids=list(range(NCORES)), trace=trace)
    acc = np.zeros((E, BT), dtype=np.float32)
    for out_map in res.results:
        acc += out_map["yT"]
    y = np.ascontiguousarray(acc.T).reshape(B, T, E).astype(np.float32)
    return y, res


def kernel(**inputs):
    y, _ = run(inputs)
    return y


# revision 25
# speedup vs baseline: 305.4591x; 305.4591x over previous
"""Causal self-attention (GPT-2 style) on 8 Trainium2 NeuronCores.

Sharding: tensor-parallel over heads. Each of the 8 cores owns 2 of the 16
heads: it computes the q/k/v projections for its heads (column-sharded
w_attn), runs causal attention for them, and multiplies by its row-slice of
w_proj, producing a partial (B*T, E) output. The host sums the 8 partials.

Layout trick: the host feeds X pre-transposed (xT = X.T, [E, B*T]) so every
matmul on-device has its contraction dim on partitions with no on-device
transposes of X. Attention is computed in the S^T = K @ Q^T layout
([s, ti] tiles): softmax denominators come for free from a ones-column
appended to V (row 64 of the AV accumulator), and normalization is applied
to the 64-row attn-out^T slabs. exp() is applied without max-subtraction:
scores for this distribution are O(1) (softmax is shift-invariant; the
reference's masked lanes underflow to exactly 0 the same way). Causal
masking adds -1e9 (underflowing exp to +0) via a wide sliding-window mask
so each E tile has a single producer engine.

Matmuls use float32r operands (full-rate on the PE at N>=256 vs 4x slower
plain fp32); operand tensors are declared float32r end-to-end so DMAs are
passthrough and compute producers round on write. Built on bacc.Bacc +
compile() so multi-wait instructions get legalized (generate_event_semaphores
splits them; raw Bass hits walrus "Too many sync wait commands").
"""

import numpy as np
from contextlib import ExitStack

import concourse.bass as bass
import concourse.bacc as bacc
import concourse.mybir as mybir
import concourse.tile as tile
from concourse import bass_utils

F32 = mybir.dt.float32
F32R = mybir.dt.float32r
AF = mybir.ActivationFunctionType

B, T, E = 2, 2048, 1024
NH, DH = 16, 64
NCORES = 8
HPC = NH // NCORES          # heads per core = 2
BT = B * T                  # 4096 tokens total
TCH = 512                   # token chunk (matmul moving size / PSUM bank)
NTC = BT // TCH             # 8 token chunks
NE = E // 128               # 8 contraction tiles over E
ST = T // 128               # 16 key tiles per batch
CPB = T // TCH              # 4 query chunks per batch
SCALE = 1.0 / 8.0           # 1/sqrt(DH)


def _kernel_body(ctx: ExitStack, tc: tile.TileContext, yT, xT, wqkv, bqkv,
                 wproj, bproj, maskd, identd, onesd):
    nc = tc.nc

    singles = ctx.enter_context(tc.tile_pool(name="singles", bufs=1))
    xpool = ctx.enter_context(tc.tile_pool(name="xpool", bufs=3))
    vtp = ctx.enter_context(tc.tile_pool(name="vtp", bufs=3))
    epool = ctx.enter_context(tc.tile_pool(name="epool", bufs=6))
    rpool = ctx.enter_context(tc.tile_pool(name="rpool", bufs=3))
    ypool = ctx.enter_context(tc.tile_pool(name="ypool", bufs=4))
    psum = ctx.enter_context(tc.tile_pool(name="psum", space="PSUM", bufs=2))

    # --- constants / persistent buffers ---
    wsb = singles.tile([128, NE, 3 * 128], F32R)
    nc.sync.dma_start(out=wsb, in_=wqkv.rearrange("(e p) m -> p e m", p=128))
    bq_sb = singles.tile([128, 3], F32)
    nc.sync.dma_start(out=bq_sb, in_=bqkv.rearrange("(c p) -> p c", p=128))
    wp_sb = singles.tile([128, E], F32R)
    nc.sync.dma_start(out=wp_sb, in_=wproj)
    bp_sb = singles.tile([128, NE], F32)
    nc.sync.dma_start(out=bp_sb, in_=bproj.rearrange("(c p) -> p c", p=128))
    mask_sb = singles.tile([128, 2 * TCH], F32)
    nc.sync.dma_start(out=mask_sb, in_=maskd)
    # stacked identity: rows 0-63 = I64, rows 64-127 = I64, so a slice with
    # any 64-aligned base partition is available for PE transposes
    id_sb = singles.tile([128, 64], F32)
    nc.sync.dma_start(out=id_sb, in_=identd)

    ones_sb = singles.tile([1, 64], F32R)
    nc.sync.dma_start(out=ones_sb, in_=onesd[0:1, :])

    qT = singles.tile([128, BT], F32R)   # rows: 2 heads x 64 dh
    kT = singles.tile([128, BT], F32R)
    aoT = singles.tile([128, BT], F32R)  # normalized attn-out^T
    # V in natural [s, dh] layout per (batch, head, s-tile), with a ones
    # column at index 64 (computes the softmax denominator inside AV).
    v1 = singles.tile([128, B, HPC, ST, 65], F32R)
    nc.sync.dma_start(
        out=v1[:, :, :, :, 64:65],
        in_=onesd.rearrange("p (b h s) -> p b h s", b=B, h=HPC)[:, :, :, :, None])

    # --- phase A: qkv^T = wqkv^T @ x^T, plus V tile transposes ---
    for tcx in range(NTC):
        bidx = tcx // CPB
        xch = xpool.tile([128, NE, TCH], F32R)
        nc.sync.dma_start(
            out=xch,
            in_=xT.rearrange("(e p) t -> p e t", p=128)[
                :, :, tcx * TCH:(tcx + 1) * TCH],
        )
        for m in range(3):
            ps = psum.tile([128, TCH], F32, tag="mm512", bufs=4, name=f"psA{tcx}_{m}")
            for e in range(NE):
                nc.tensor.matmul(
                    ps,
                    lhsT=wsb[:, e, m * 128:(m + 1) * 128],
                    rhs=xch[:, e, :],
                    start=(e == 0),
                    stop=(e == NE - 1),
                )
            if m == 0:
                nc.vector.tensor_scalar_add(
                    qT[:, tcx * TCH:(tcx + 1) * TCH], ps, bq_sb[:, 0:1])
            elif m == 1:
                nc.vector.tensor_scalar_add(
                    kT[:, tcx * TCH:(tcx + 1) * TCH], ps, bq_sb[:, 1:2])
            else:
                vtile = vtp.tile([128, TCH], F32)
                nc.scalar.activation(vtile, ps, AF.Identity, bias=bq_sb[:, 2:3])
                for hh in range(HPC):
                    for ss in range(TCH // 128):
                        s_idx = (tcx % CPB) * (TCH // 128) + ss
                        ps_t = psum.tile([128, 64], F32, tag="aux", bufs=2,
                                         name=f"pst{tcx}_{hh}_{ss}")
                        nc.tensor.transpose(
                            ps_t,
                            vtile[hh * 64:(hh + 1) * 64, ss * 128:(ss + 1) * 128],
                            id_sb[hh * 64:(hh + 1) * 64, :],
                        )
                        nc.scalar.copy(
                            v1[:, bidx, hh, s_idx, 0:64], ps_t)

    # --- phase B: causal attention per (batch, head) in S^T layout ---
    for bidx in range(B):
        for hh in range(HPC):
            hs = slice(hh * 64, (hh + 1) * 64)
            for c in range(CPB):
                tis = slice(bidx * T + c * TCH, bidx * T + (c + 1) * TCH)
                out_ps = psum.tile([65, TCH], F32, tag="out65", bufs=2,
                                   name=f"outp{bidx}_{hh}_{c}")
                smax = 4 * c + 3
                for s in range(smax + 1):
                    s_ps = psum.tile([128, TCH], F32, tag="mm512", bufs=4,
                                     name=f"psS{bidx}_{hh}_{c}_{s}")
                    nc.tensor.matmul(
                        s_ps,
                        lhsT=kT[hs, bidx * T + s * 128:bidx * T + (s + 1) * 128],
                        rhs=qT[hs, tis],
                        start=True, stop=True,
                    )
                    et = epool.tile([128, TCH], F32R)
                    if s >= 4 * c:
                        # additive causal mask (-1e9 where ti < s) in PSUM,
                        # then one exp: E keeps a single producer engine
                        off = s * 128 - c * TCH
                        nc.vector.tensor_add(
                            s_ps, s_ps, mask_sb[:, TCH - off:2 * TCH - off])
                    nc.scalar.activation(et, s_ps, AF.Exp, scale=SCALE)
                    nc.tensor.matmul(
                        out_ps,
                        lhsT=v1[:, bidx, hh, s, :],
                        rhs=et,
                        start=(s == 0), stop=(s == smax),
                    )
                # normalize: rows 0..63 are unnormalized out^T, row 64 = denom
                r = rpool.tile([1, TCH], F32R)
                nc.vector.reciprocal(r, out_ps[64:65, :])
                rb_ps = psum.tile([64, TCH], F32, tag="aux", bufs=2,
                                  name=f"rbp{bidx}_{hh}_{c}")
                nc.tensor.matmul(rb_ps, lhsT=ones_sb,
                                 rhs=r, start=True, stop=True)
                rbs = rpool.tile([64, TCH], F32)
                nc.scalar.copy(rbs, rb_ps)
                nc.vector.tensor_mul(aoT[hs, tis], out_ps[0:64, :], rbs)

    # --- phase C: partial y^T = wproj^T @ attn_out^T (+ bias on core 0) ---
    for oc in range(NE):
        for tc2 in range(NTC):
            ps = psum.tile([128, TCH], F32, tag="mm512", bufs=4,
                           name=f"psC{oc}_{tc2}")
            nc.tensor.matmul(
                ps,
                lhsT=wp_sb[:, oc * 128:(oc + 1) * 128],
                rhs=aoT[:, tc2 * TCH:(tc2 + 1) * TCH],
                start=True, stop=True,
            )
            ysb = ypool.tile([128, TCH], F32)
            nc.scalar.activation(ysb, ps, AF.Identity, bias=bp_sb[:, oc:oc + 1])
            nc.sync.dma_start(
                out=yT[oc * 128:(oc + 1) * 128, tc2 * TCH:(tc2 + 1) * TCH],
                in_=ysb)


def build_bass():
    nc = bacc.Bacc("TRN2", target_bir_lowering=False, debug=False,
                   enable_asserts=False, num_devices=NCORES)
    xT = nc.dram_tensor("xT", [E, BT], F32R, kind="ExternalInput").ap()
    wqkv = nc.dram_tensor("wqkv", [E, 3 * 128], F32R, kind="ExternalInput").ap()
    bqkv = nc.dram_tensor("bqkv", [3 * 128], F32, kind="ExternalInput").ap()
    wproj = nc.dram_tensor("wproj", [128, E], F32R, kind="ExternalInput").ap()
    bproj = nc.dram_tensor("bproj", [E], F32, kind="ExternalInput").ap()
    maskd = nc.dram_tensor("maskd", [128, 2 * TCH], F32, kind="ExternalInput").ap()
    identd = nc.dram_tensor("identd", [128, 64], F32, kind="ExternalInput").ap()
    onesd = nc.dram_tensor("onesd", [128, 64], F32R, kind="ExternalInput").ap()
    yT = nc.dram_tensor("yT", [E, BT], F32, kind="ExternalOutput").ap()
    with tile.TileContext(nc) as tc:
        with nc.allow_low_precision(reason="fp32r matmul operand production"):
            with ExitStack() as ctx:
                _kernel_body(ctx, tc, yT, xT, wqkv, bqkv, wproj, bproj, maskd,
                             identd, onesd)
    nc.compile()
    return nc


def make_in_maps(inputs):
    stacked = np.asarray(inputs["stacked"], dtype=np.float32)
    w_attn = np.asarray(inputs["w_attn"], dtype=np.float32)
    b_attn = np.asarray(inputs["b_attn"], dtype=np.float32)
    w_proj = np.asarray(inputs["w_proj"], dtype=np.float32)
    b_proj = np.asarray(inputs["b_proj"], dtype=np.float32)

    xT = np.ascontiguousarray(stacked.reshape(BT, E).T)
    # W[r, w] = 0 where (w - TCH) >= r else -1e9; sliced per diagonal offset
    ww = np.arange(2 * TCH)[None, :] - TCH
    rr = np.arange(128)[:, None]
    mask = np.where(ww >= rr, 0.0, -1e9).astype(np.float32)
    ident = np.concatenate(
        [np.eye(64, dtype=np.float32), np.eye(64, dtype=np.float32)], axis=0)

    in_maps = []
    for c in range(NCORES):
        lo = c * HPC * DH
        hi = lo + HPC * DH
        wq = np.concatenate(
            [w_attn[:, lo:hi], w_attn[:, E + lo:E + hi],
             w_attn[:, 2 * E + lo:2 * E + hi]], axis=1)
        bq = np.concatenate(
            [b_attn[lo:hi], b_attn[E + lo:E + hi], b_attn[2 * E + lo:2 * E + hi]])
        in_maps.append({
            "xT": xT,
            "wqkv": np.ascontiguousarray(wq),
            "bqkv": np.ascontiguousarray(bq),
            "wproj": np.ascontiguousarray(w_proj[lo:hi, :]),
            "bproj": b_proj if c == 0 else np.zeros_like(b_proj),
            "maskd": mask,
            "identd": ident,
            "onesd": np.ones((128, 64), dtype=np.float32),
        })
    return in_maps


_NC = None


def _get_nc():
    global _NC
    if _NC is None:
        _NC = build_bass()
    return _NC


def run(inputs, trace=False):
    nc = _get_nc()
    in_maps = make_in_maps(inputs)
    res = bass_utils.run_bass_kernel_spmd(
        nc, in_maps, core_ids=list(range(NCORES)), trace=trace)
    acc = np.zeros((E, BT), dtype=np.float32)
    for out_map in res.results:
        acc += out_map["yT"]
    y = np.ascontiguousarray(acc.T).reshape(B, T, E).astype(np.float32)
    return y, res


def kernel(**inputs):
    y, _ = run(inputs)
    return y
